# revision 1
# baseline (speedup 1.0000x reference)
import sys

sys.path.insert(0, "/opt/trn_rl_repo")
import numpy as np
import concourse.bass as bass
import concourse.bacc as bacc
import concourse.mybir as mybir
import concourse.tile as tile
from concourse import masks
import concourse.bass_utils as bass_utils

bass_utils.upload_artifacts = lambda tmpdir: "local://" + tmpdir
from concourse.bass_utils import run_bass_kernel_spmd

N_CORES = 8
B, H, W, C, R = 32, 56, 56, 256, 16
BS = B // N_CORES          # 4 samples per core
NP = H * W                 # 3136 pixels per sample
NT = 25                    # tiles per sample: 24 x 128 + 1 x 64
ROWS = BS * NP             # 12544 rows per core
F32 = mybir.dt.float32
AL = mybir.AluOpType
AF = mybir.ActivationFunctionType
AX = mybir.AxisListType

_COMPILED = None


def _build():
    nc = bacc.Bacc(None, target_bir_lowering=False, num_devices=N_CORES)
    x_d = nc.declare_dram_parameter("x", [ROWS, C], F32, isOutput=False)
    w1_d = nc.declare_dram_parameter("w1", [C, R], F32, isOutput=False)
    b1_d = nc.declare_dram_parameter("b1", [1, R], F32, isOutput=False)
    w2_d = nc.declare_dram_parameter("w2", [R, C], F32, isOutput=False)
    b2_d = nc.declare_dram_parameter("b2", [1, C], F32, isOutput=False)
    wf_d = nc.declare_dram_parameter("wflat", [98, 1], F32, isOutput=False)
    bc_d = nc.declare_dram_parameter("bconv", [1, 1], F32, isOutput=False)
    out_d = nc.declare_dram_parameter("out", [ROWS, C], F32, isOutput=True)

    flat_dram = nc.dram_tensor("flat_dram", [2 * BS, NP], F32)
    fpad_dram = nc.dram_tensor("fpad_dram", [2 * BS, 3844], F32)

    with tile.TileContext(nc) as tc:
        with tc.tile_pool(name="const", bufs=1) as cp, \
             tc.tile_pool(name="xbuf", bufs=1) as xp, \
             tc.tile_pool(name="work", bufs=3) as wp, \
             tc.tile_pool(name="sp", bufs=3) as spp, \
             tc.tile_pool(name="psA", bufs=2, space="PSUM") as psA, \
             tc.tile_pool(name="psB", bufs=3, space="PSUM") as psB, \
             tc.tile_pool(name="psC", bufs=3, space="PSUM") as psC:

            # ---------- constants ----------
            ident = cp.tile([128, 128], F32)
            masks.make_identity(nc, ident[:])
            ones2 = cp.tile([2, 128], F32)
            nc.gpsimd.memset(ones2[:], 1.0)

            w1t = cp.tile([128, 2 * R], F32)       # [K-chunk, 2*16]
            nc.sync.dma_start(w1t[:, 0:R], w1_d[0:128, :])
            nc.sync.dma_start(w1t[:, R:2 * R], w1_d[128:256, :])
            w2t = cp.tile([R, C], F32)
            nc.sync.dma_start(w2t[:], w2_d[:])
            wf_t = cp.tile([98, 1], F32)
            nc.sync.dma_start(wf_t[:], wf_d[:])

            b1r = cp.tile([1, R], F32)
            nc.sync.dma_start(b1r[:], b1_d[:])
            b1b = cp.tile([2, R], F32)
            nc.gpsimd.partition_broadcast(b1b[:], b1r[:], channels=2)
            b2r = cp.tile([1, C], F32)
            nc.sync.dma_start(b2r[:], b2_d[:])
            b2b = cp.tile([2, C], F32)
            nc.gpsimd.partition_broadcast(b2b[:], b2r[:], channels=2)
            bcr = cp.tile([1, 1], F32)
            nc.sync.dma_start(bcr[:], bc_d[:])
            bcb = cp.tile([128, 1], F32)
            nc.gpsimd.partition_broadcast(bcb[:], bcr[:], channels=128)

            # zero the padded-plane dram scratch (borders stay zero forever)
            zrow = cp.tile([2 * BS, 3844], F32)
            nc.vector.memset(zrow[:], 0.0)
            nc.sync.dma_start(fpad_dram.ap(), zrow[:])

            # resident x (overwritten in place by xg then by out)
            xbuf = xp.tile([128, BS * NT * C], F32)

            def xt(s, t):
                pt = 64 if t == NT - 1 else 128
                return xbuf[0:pt, (s * NT + t) * C:(s * NT + t + 1) * C]

            def xrows(s, t):
                r0 = s * NP + t * 128
                pt = 64 if t == NT - 1 else 128
                return x_d[r0:r0 + pt, :], out_d[r0:r0 + pt, :]

            maxacc_l, rhs_l, cb_l, spm_l, spx_l, spsc_l = {}, {}, {}, {}, {}, {}

            for s in range(BS):
                for t in range(NT):
                    src, _ = xrows(s, t)
                    nc.sync.dma_start(xt(s, t), src)

            for s in range(BS):
                # ============ phase A: load + pooling stats ============
                acc0 = psA.tile([128, 128], F32, tag="acc")
                acc1 = psA.tile([128, 128], F32, tag="acc")
                accs = [acc0, acc1]
                maxacc = wp.tile([128, C], F32, tag="maxacc")
                for t in range(NT):
                    pt = 64 if t == NT - 1 else 128
                    v = xt(s, t)
                    for c in range(2):
                        nc.tensor.matmul(
                            accs[c][:, 0:pt], v[:, c * 128:(c + 1) * 128],
                            ident[0:pt, 0:pt], is_transpose=True,
                            start=(t == 0), stop=(t == NT - 1),
                            skip_group_check=True)
                    if t == 0:
                        nc.vector.tensor_copy(maxacc[:], v)
                    else:
                        nc.vector.tensor_tensor(
                            out=maxacc[0:pt, :], in0=maxacc[0:pt, :], in1=v,
                            op=AL.max)

                # ============ phase A finalize: pooled vectors ============
                rhs_s = wp.tile([128, 4], F32, tag="rhs")
                for c in range(2):
                    tmp = wp.tile([128, 1], F32, tag="redtmp")
                    nc.vector.reduce_sum(tmp[:], accs[c][:], axis=AX.X)
                    nc.scalar.activation(rhs_s[:, 2 * c:2 * c + 1], tmp[:],
                                         AF.Copy, scale=1.0 / NP)
                    mt = psB.tile([128, 128], F32, tag="psb")
                    nc.tensor.transpose(mt[:], maxacc[:, c * 128:(c + 1) * 128],
                                        ident[:])
                    nc.vector.reduce_max(rhs_s[:, 2 * c + 1:2 * c + 2], mt[:],
                                         axis=AX.X)

                # ============ phase B: MLP -> channel scale row ============
                h_ps = psB.tile([2, R], F32, tag="psb")
                nc.tensor.matmul(h_ps[:], rhs_s[:, 0:2], w1t[:, 0:R],
                                 start=True, stop=False)
                nc.tensor.matmul(h_ps[:], rhs_s[:, 2:4], w1t[:, R:2 * R],
                                 start=False, stop=True)
                hb = wp.tile([2, R], F32, tag="hb")
                nc.vector.tensor_tensor(out=hb[:], in0=h_ps[:], in1=b1b[:],
                                        op=AL.add)
                hr = wp.tile([2, R], F32, tag="hr")
                nc.scalar.activation(hr[:], hb[:], AF.Relu)
                hT_ps = psB.tile([R, 2], F32, tag="psb")
                nc.tensor.transpose(hT_ps[:], hr[:], ident[0:2, 0:2])
                hT = wp.tile([R, 2], F32, tag="hT")
                nc.vector.tensor_copy(hT[:], hT_ps[:])
                co_ps = psB.tile([2, C], F32, tag="psb")
                nc.tensor.matmul(co_ps[:], hT[:], w2t[:], start=True, stop=True)
                co_sb = wp.tile([2, C], F32, tag="co")
                nc.vector.tensor_tensor(out=co_sb[:], in0=co_ps[:], in1=b2b[:],
                                        op=AL.add)
                sig = wp.tile([2, C], F32, tag="sig")
                nc.scalar.activation(sig[:], co_sb[:], AF.Sigmoid)
                cb_ps = psB.tile([128, C], F32, tag="psb")
                nc.tensor.matmul(cb_ps[:], ones2[:], sig[:], start=True, stop=True)
                cb = wp.tile([128, C], F32, tag="cb")
                nc.vector.tensor_copy(cb[:], cb_ps[:])

                # ============ phase C: xg (in place) + spatial stats ============
                spm = spp.tile([128, NT], F32, tag="spm")
                spx = spp.tile([128, NT], F32, tag="spx")
                nc.vector.memset(spm[64:128, NT - 1:NT], 0.0)
                nc.vector.memset(spx[64:128, NT - 1:NT], 0.0)
                for t in range(NT):
                    pt = 64 if t == NT - 1 else 128
                    v = xt(s, t)
                    nc.vector.tensor_tensor(out=v, in0=v, in1=cb[0:pt, :],
                                            op=AL.mult)
                    nc.vector.reduce_max(spx[0:pt, t:t + 1], v, axis=AX.X)
                    nc.scalar.activation(v, v, AF.Copy,
                                         accum_out=spm[0:pt, t:t + 1])

                # ============ phase D: 7x7x2 conv via patch matmuls ============
                for c, sp_t in enumerate((spm, spx)):
                    row = s * 2 + c
                    spT_ps = psB.tile([NT, 128], F32, tag="psb")
                    nc.tensor.transpose(spT_ps[:], sp_t[:], ident[:])
                    spT = wp.tile([NT, 128], F32, tag="spT")
                    nc.vector.tensor_copy(spT[:], spT_ps[:])
                    nc.sync.dma_start(
                        bass.AP(flat_dram, row * NP, [[128, 24], [1, 128]]),
                        spT[0:24, :])
                    nc.sync.dma_start(
                        bass.AP(flat_dram, row * NP + 3072, [[1, 64]]),
                        spT[24:25, 0:64])
                    nc.sync.dma_start(
                        bass.AP(fpad_dram, row * 3844 + 3 * 62 + 3,
                                [[62, 56], [1, 56]]),
                        bass.AP(flat_dram, row * NP, [[56, 56], [1, 56]]))
                patches = wp.tile([98, NP], F32, tag="patches")
                for c in range(2):
                    row = s * 2 + c
                    for dy in range(7):
                        nc.sync.dma_start(
                            patches[c * 49 + dy * 7:c * 49 + dy * 7 + 7, :],
                            bass.AP(fpad_dram, row * 3844 + dy * 62,
                                    [[1, 7], [62, 56], [1, 56]]))
                conv_ps = psC.tile([128, NT], F32, tag="conv")
                for t in range(NT):
                    pt = 64 if t == NT - 1 else 128
                    nc.tensor.matmul(conv_ps[0:pt, t:t + 1],
                                     patches[:, t * 128:t * 128 + pt],
                                     wf_t[:], start=True, stop=True,
                                     skip_group_check=True)
                nc.vector.memset(conv_ps[64:128, NT - 1:NT], 0.0)
                spsc = spp.tile([128, NT], F32, tag="spsc")
                nc.scalar.activation(spsc[:], conv_ps[:], AF.Sigmoid, bias=bcb[:])

                # ============ phase E: out = xg * spatial, store ============
                for t in range(NT):
                    pt = 64 if t == NT - 1 else 128
                    v = xt(s, t)
                    _, dst = xrows(s, t)
                    nc.scalar.activation(v, v, AF.Copy,
                                         scale=spsc[0:pt, t:t + 1])
                    nc.sync.dma_start(dst, v)

    nc.compile()
    return nc


def _get_compiled():
    global _COMPILED
    if _COMPILED is None:
        _COMPILED = _build()
    return _COMPILED


def kernel(x, w1, b1, w2, b2, wconv, bconv):
    x = np.ascontiguousarray(np.asarray(x, dtype=np.float32))
    # wconv [7,7,2,1] -> wflat[k] = wconv[dy,dx,c]; k = c*49 + dy*7 + dx
    wf = np.asarray(wconv, dtype=np.float32)[:, :, :, 0].transpose(2, 0, 1).copy()
    wf[0] /= C          # fold the channel-mean (1/256) into the conv weight
    wf = wf.reshape(98, 1)

    nc = _get_compiled()
    xs = x.reshape(N_CORES, ROWS, C)
    in_maps = [{
        "x": xs[i],
        "w1": np.asarray(w1, np.float32),
        "b1": np.asarray(b1, np.float32).reshape(1, R),
        "w2": np.asarray(w2, np.float32),
        "b2": np.asarray(b2, np.float32).reshape(1, C),
        "wflat": wf,
        "bconv": np.asarray(bconv, np.float32).reshape(1, 1),
    } for i in range(N_CORES)]
    res = run_bass_kernel_spmd(nc, in_maps, list(range(N_CORES)))
    out = np.stack([res.results[i]["out"] for i in range(N_CORES)], axis=0)
    return out.reshape(B, H, W, C)



# revision 4
# speedup vs baseline: 1.7951x; 1.7951x over previous
import sys

sys.path.insert(0, "/opt/trn_rl_repo")
import numpy as np
import concourse.bass as bass
import concourse.bacc as bacc
import concourse.mybir as mybir
import concourse.tile as tile
from concourse import masks
import concourse.bass_utils as bass_utils

bass_utils.upload_artifacts = lambda tmpdir: "local://" + tmpdir
from concourse.bass_utils import run_bass_kernel_spmd

N_CORES = 8
B, H, W, C, R = 32, 56, 56, 256, 16
BS = B // N_CORES          # 4 samples per core
NP = H * W                 # 3136 pixels per sample
NT = 25                    # tiles per sample: 24 x 128 + 1 x 64
ROWS = BS * NP             # 12544 rows per core
F32 = mybir.dt.float32
BF16 = mybir.dt.bfloat16
AL = mybir.AluOpType
AF = mybir.ActivationFunctionType
AX = mybir.AxisListType

_COMPILED = None


def _build():
    nc = bacc.Bacc(None, target_bir_lowering=False, num_devices=N_CORES)
    x_d = nc.declare_dram_parameter("x", [ROWS, C], F32, isOutput=False)
    w1_d = nc.declare_dram_parameter("w1s", [128, 64], F32, isOutput=False)
    w2_d = nc.declare_dram_parameter("w2", [R, C], F32, isOutput=False)
    b1_d = nc.declare_dram_parameter("b1c", [R, 1], F32, isOutput=False)
    b2_d = nc.declare_dram_parameter("b2r", [1, C], F32, isOutput=False)
    tp_d = nc.declare_dram_parameter("tp", [62, 784], F32, isOutput=False)
    bc_d = nc.declare_dram_parameter("bconv", [1, 1], F32, isOutput=False)
    out_d = nc.declare_dram_parameter("out", [ROWS, C], F32, isOutput=True)

    flat_dram = nc.dram_tensor("flat_dram", [2 * BS, NP], F32)
    sflat_dram = nc.dram_tensor("sflat_dram", [BS, NP], F32)

    with tile.TileContext(nc) as tc:
        with tc.tile_pool(name="const", bufs=1) as cp, \
             tc.tile_pool(name="xbuf", bufs=1) as xp, \
             tc.tile_pool(name="work", bufs=3) as wp, \
             tc.tile_pool(name="sp", bufs=2) as spp, \
             tc.tile_pool(name="psA", bufs=2, space="PSUM") as psA, \
             tc.tile_pool(name="psB", bufs=2, space="PSUM") as psB, \
             tc.tile_pool(name="psC", bufs=2, space="PSUM") as psC:

            # ---------- constants ----------
            ident = cp.tile([128, 128], F32)
            masks.make_identity(nc, ident[:])
            ones_col = cp.tile([128, 1], BF16)
            nc.gpsimd.memset(ones_col[:], 1.0)
            ones1 = cp.tile([1, 128], F32)
            nc.gpsimd.memset(ones1[:], 1.0)

            w1t = cp.tile([128, 64], F32)     # [c-chunk, (avg0 avg1 max0 max1)x16]
            nc.sync.dma_start(w1t[:], w1_d[:])
            w2t = cp.tile([R, C], F32)
            nc.sync.dma_start(w2t[:], w2_d[:])
            b1c = cp.tile([R, 1], F32)
            nc.sync.dma_start(b1c[:], b1_d[:])
            b2r = cp.tile([1, C], F32)
            nc.sync.dma_start(b2r[:], b2_d[:])
            tpt = cp.tile([62, 784], F32)     # 14 Toeplitz mats [62,56]
            nc.sync.dma_start(tpt[:], tp_d[:])
            bcr = cp.tile([1, 1], F32)
            nc.sync.dma_start(bcr[:], bc_d[:])
            bcb = cp.tile([56, 1], F32)
            nc.gpsimd.partition_broadcast(bcb[:], bcr[:], channels=56)

            plane_m = cp.tile([62, 62], F32)
            plane_x = cp.tile([62, 62], F32)
            nc.vector.memset(plane_m[:], 0.0)
            nc.vector.memset(plane_x[:], 0.0)

            # resident x in bf16 (becomes xg, then out, in place)
            xbuf = xp.tile([128, BS * NT * C], BF16)

            for s in range(BS):
                s0 = s * NT * C
                # garbage rows of the 64-row tail tile stay zero throughout
                nc.vector.memset(xbuf[64:128, s0 + 24 * C:s0 + 25 * C], 0.0)

            for s in range(BS):
                s0 = s * NT * C
                vfull = xbuf[:, s0:s0 + NT * C]
                vmain = xbuf[:, s0:s0 + 24 * C]
                vtail = xbuf[0:64, s0 + 24 * C:s0 + 25 * C]

                # ---------- load (cast fp32 -> bf16 during DMA) ----------
                nc.gpsimd.dma_start(
                    vmain, bass.AP(x_d, s * NP * C, [[256, 128], [32768, 24], [1, 256]]))
                nc.gpsimd.dma_start(
                    vtail, bass.AP(x_d, (s * NP + 3072) * C, [[256, 64], [1, 256]]))

                # ---------- phase A: pooling stats ----------
                bankA = psA.tile([128, 512], F32, tag="bankA")
                prow_ps = bankA[0:1, 0:C]
                for t in range(NT):
                    nc.tensor.matmul(
                        prow_ps, ones_col[:],
                        xbuf[:, s0 + t * C:s0 + (t + 1) * C],
                        start=(t == 0), stop=(t == NT - 1),
                        skip_group_check=True)

                maxacc = wp.tile([128, C], F32, tag="maxacc")
                nc.vector.tensor_reduce(
                    maxacc[:], vmain.rearrange("p (t c) -> p c t", c=C),
                    axis=AX.X, op=AL.max)
                nc.vector.tensor_tensor(
                    out=maxacc[0:64, :], in0=maxacc[0:64, :], in1=vtail, op=AL.max)

                prow = wp.tile([1, C], F32, tag="prow_sb")
                nc.scalar.copy(prow[:], prow_ps)
                pcols_ps = bankA[0:128, 256:260]
                nc.tensor.transpose(pcols_ps[:, 0:1], prow[0:1, 0:128],
                                    ident[0:1, 0:1])
                nc.tensor.transpose(pcols_ps[:, 1:2], prow[0:1, 128:256],
                                    ident[0:1, 0:1])
                bankB = psB.tile([128, 512], F32, tag="bankB")
                mt0_ps = bankB[0:128, 0:128]
                nc.tensor.transpose(mt0_ps, maxacc[:, 0:128], ident[:])
                mt1_ps = bankB[0:128, 128:256]
                nc.tensor.transpose(mt1_ps, maxacc[:, 128:256], ident[:])
                pcols = wp.tile([128, 4], F32, tag="pcols_sb")
                nc.scalar.copy(pcols[:, 0:2], pcols_ps[:, 0:2])
                nc.vector.reduce_max(pcols[:, 2:3], mt0_ps, axis=AX.X)
                nc.vector.reduce_max(pcols[:, 3:4], mt1_ps, axis=AX.X)

                # ---------- phase B: MLP -> per-channel scale ----------
                hT_ps = bankA[0:R, 260:262]
                for p in range(2):          # 0 = avg (w1/NP), 1 = max
                    for k in range(2):      # channel chunk
                        nc.tensor.matmul(
                            hT_ps[:, p:p + 1],
                            w1t[:, (p * 2 + k) * R:(p * 2 + k + 1) * R],
                            pcols[:, 2 * p + k:2 * p + k + 1],
                            start=(k == 0), stop=(k == 1),
                            skip_group_check=True)
                hr = wp.tile([R, 2], F32, tag="hr")
                nc.scalar.activation(hr[:], hT_ps, AF.Relu, bias=b1c[:])
                co_ps = bankA[0:1, 0:2 * C]
                nc.tensor.matmul(co_ps[:, 0:C], hr[:, 0:1], w2t[:],
                                 start=True, stop=True)
                nc.tensor.matmul(co_ps[:, C:2 * C], hr[:, 1:2], w2t[:],
                                 start=True, stop=True)
                sg0 = wp.tile([1, C], F32, tag="sg0")
                nc.vector.tensor_tensor(out=sg0[:], in0=co_ps[0:1, 0:C],
                                        in1=b2r[:], op=AL.add)
                sg1 = wp.tile([1, C], F32, tag="sg1")
                nc.vector.tensor_tensor(out=sg1[:], in0=co_ps[0:1, C:2 * C],
                                        in1=b2r[:], op=AL.add)
                sgs0 = wp.tile([1, C], F32, tag="sgs0")
                nc.scalar.activation(sgs0[:], sg0[:], AF.Sigmoid)
                sgs1 = wp.tile([1, C], F32, tag="sgs1")
                nc.scalar.activation(sgs1[:], sg1[:], AF.Sigmoid)
                bankC = psC.tile([128, 512], F32, tag="bankC")
                cb_ps = bankC[0:128, 0:C]
                nc.tensor.matmul(cb_ps, ones1[:], sgs0[:],
                                 start=True, stop=False, skip_group_check=True)
                nc.tensor.matmul(cb_ps, ones1[:], sgs1[:],
                                 start=False, stop=True, skip_group_check=True)
                cb = wp.tile([128, C], BF16, tag="cb_sb")
                nc.scalar.copy(cb[:], cb_ps)

                # ---------- phase C: xg (in place) + spatial stats ----------
                v3 = vfull.rearrange("p (t c) -> p t c", c=C)
                nc.vector.tensor_tensor(
                    out=v3, in0=v3,
                    in1=cb[:].unsqueeze(1).broadcast_to([128, NT, C]),
                    op=AL.mult)
                spx = spp.tile([128, NT], F32, tag="spx")
                nc.vector.tensor_reduce(spx[:], v3, axis=AX.X, op=AL.max)
                spm = spp.tile([128, NT], F32, tag="spm")
                nc.vector.tensor_reduce(spm[:], v3, axis=AX.X, op=AL.add)

                # ---------- phase D: 7x7x2 conv via Toeplitz matmuls ----------
                for c, (sp_t, plane) in enumerate(
                        ((spm, plane_m), (spx, plane_x))):
                    row = s * 2 + c
                    spT_ps = bankB[0:NT, 256 + c * 128:256 + (c + 1) * 128]
                    nc.tensor.transpose(spT_ps, sp_t[:], ident[:])
                    spT = wp.tile([NT, 128], F32, tag="spT_sb")
                    nc.scalar.copy(spT[:], spT_ps)
                    nc.sync.dma_start(
                        bass.AP(flat_dram, row * NP, [[128, 24], [1, 128]]),
                        spT[0:24, :])
                    nc.sync.dma_start(
                        bass.AP(flat_dram, row * NP + 3072, [[1, 64]]),
                        spT[24:25, 0:64])
                    nc.sync.dma_start(
                        plane[3:59, 3:59],
                        bass.AP(flat_dram, row * NP, [[56, 56], [1, 56]]))

                conv_ps = bankC[0:56, 256:312]
                for c, plane in enumerate((plane_m, plane_x)):
                    for dx in range(7):
                        j = (c * 7 + dx) * 56
                        nc.tensor.matmul(
                            conv_ps, tpt[:, j:j + 56], plane[0:62, dx:dx + 56],
                            start=(c == 0 and dx == 0),
                            stop=(c == 1 and dx == 6),
                            skip_group_check=True)
                s_plane = wp.tile([56, 56], F32, tag="splane")
                nc.scalar.activation(s_plane[:], conv_ps, AF.Sigmoid,
                                     bias=bcb[:])
                nc.sync.dma_start(
                    bass.AP(sflat_dram, s * NP, [[1, NP]]), s_plane[:])
                spscT = wp.tile([NT, 128], F32, tag="spscT")
                nc.sync.dma_start(spscT[0:24, :],
                                  bass.AP(sflat_dram, s * NP, [[1, 3072]]))
                nc.sync.dma_start(spscT[24:25, 0:64],
                                  bass.AP(sflat_dram, s * NP + 3072, [[1, 64]]))
                spsc_ps = bankC[0:128, 312:312 + NT]
                nc.tensor.transpose(spsc_ps, spscT[:], ident[0:NT, 0:NT])
                spsc = spp.tile([128, NT], F32, tag="spsc_sb")
                nc.scalar.copy(spsc[:], spsc_ps)

                # ---------- phase E: out = xg * spatial (split engines) ----------
                NSC = 12     # tiles 0..11 on scalar, 12..24 one vector op
                for t in range(NSC):
                    vt = xbuf[:, s0 + t * C:s0 + (t + 1) * C]
                    nc.scalar.activation(vt, vt, AF.Copy,
                                         scale=spsc[:, t:t + 1])
                vrest = xbuf[:, s0 + NSC * C:s0 + NT * C].rearrange(
                    "p (t c) -> p t c", c=C)
                nc.vector.tensor_tensor(
                    out=vrest, in0=vrest,
                    in1=spsc[:, NSC:NT].unsqueeze(2).broadcast_to(
                        [128, NT - NSC, C]),
                    op=AL.mult)

                # ---------- store (cast bf16 -> fp32 during DMA) ----------
                nc.gpsimd.dma_start(
                    bass.AP(out_d, s * NP * C, [[256, 128], [32768, 24], [1, 256]]),
                    vmain)
                nc.gpsimd.dma_start(
                    bass.AP(out_d, (s * NP + 3072) * C, [[256, 64], [1, 256]]),
                    vtail)

    nc.compile()
    return nc


def _get_compiled():
    global _COMPILED
    if _COMPILED is None:
        _COMPILED = _build()
    return _COMPILED


def make_in_maps(x, w1, b1, w2, b2, wconv, bconv):
    x = np.ascontiguousarray(np.asarray(x, dtype=np.float32))
    w1 = np.asarray(w1, np.float32)
    w2 = np.asarray(w2, np.float32)

    # w1s: [c-chunk(128), (avg_chunk0, avg_chunk1, max_chunk0, max_chunk1) x 16]
    w1avg = w1 / NP
    w1s = np.concatenate(
        [w1avg[0:128], w1avg[128:256], w1[0:128], w1[128:256]],
        axis=1).astype(np.float32)

    # Toeplitz conv mats: tp[y', (c*7+dx)*56 + y] = wf[y'-y, dx, c]
    wf = np.asarray(wconv, np.float32)[:, :, :, 0]      # [dy, dx, c]
    wf = wf.copy()
    wf[:, :, 0] /= C          # fold channel-mean into mean-plane taps
    tp = np.zeros((2, 7, 62, 56), np.float32)
    idx = np.arange(56)
    for c in range(2):
        for dx in range(7):
            for dy in range(7):
                tp[c, dx, idx + dy, idx] = wf[dy, dx, c]
    tp = tp.transpose(2, 0, 1, 3).reshape(62, 784).copy()

    xs = x.reshape(N_CORES, ROWS, C)
    return [{
        "x": xs[i],
        "w1s": w1s,
        "w2": w2,
        "b1c": np.asarray(b1, np.float32).reshape(R, 1),
        "b2r": np.asarray(b2, np.float32).reshape(1, C),
        "tp": tp,
        "bconv": np.asarray(bconv, np.float32).reshape(1, 1),
    } for i in range(N_CORES)]


def kernel(x, w1, b1, w2, b2, wconv, bconv):
    nc = _get_compiled()
    in_maps = make_in_maps(x, w1, b1, w2, b2, wconv, bconv)
    res = run_bass_kernel_spmd(nc, in_maps, list(range(N_CORES)))
    out = np.stack([res.results[i]["out"] for i in range(N_CORES)], axis=0)
    return out.reshape(B, H, W, C)


# revision 9
# speedup vs baseline: 2.0598x; 1.1475x over previous
import sys

sys.path.insert(0, "/opt/trn_rl_repo")
import numpy as np
import concourse.bass as bass
import concourse.bacc as bacc
import concourse.mybir as mybir
import concourse.tile as tile
from concourse import masks
import concourse.bass_utils as bass_utils

bass_utils.upload_artifacts = lambda tmpdir: "local://" + tmpdir
from concourse.bass_utils import run_bass_kernel_spmd

N_CORES = 8
B, H, W, C, R = 32, 56, 56, 256, 16
BS = B // N_CORES          # 4 samples per core
NP = H * W                 # 3136 pixels per sample
NT = 25                    # tiles per sample: 24 x 128 + 1 x 64
ROWS = BS * NP             # 12544 rows per core
F32 = mybir.dt.float32
BF16 = mybir.dt.bfloat16
AL = mybir.AluOpType
AF = mybir.ActivationFunctionType
AX = mybir.AxisListType

_COMPILED = None


def _build():
    nc = bacc.Bacc(None, target_bir_lowering=False, num_devices=N_CORES)
    x_d = nc.declare_dram_parameter("x", [ROWS, C], F32, isOutput=False)
    w1_d = nc.declare_dram_parameter("w1s", [128, 64], F32, isOutput=False)
    w2_d = nc.declare_dram_parameter("w2", [R, C], F32, isOutput=False)
    b1_d = nc.declare_dram_parameter("b1c", [R, 1], F32, isOutput=False)
    b2_d = nc.declare_dram_parameter("b2r", [1, C], F32, isOutput=False)
    tp_d = nc.declare_dram_parameter("tp", [62, 784], F32, isOutput=False)
    bc_d = nc.declare_dram_parameter("bconv", [1, 1], F32, isOutput=False)
    out_d = nc.declare_dram_parameter("out", [ROWS, C], F32, isOutput=True)

    flat_dram = nc.dram_tensor("flat_dram", [2 * BS, NP], F32)
    sflat_dram = nc.dram_tensor("sflat_dram", [BS, NP], F32)

    with tile.TileContext(nc) as tc:
        with tc.tile_pool(name="const", bufs=1) as cp, \
             tc.tile_pool(name="xbuf", bufs=1) as xp, \
             tc.tile_pool(name="work", bufs=3) as wp, \
             tc.tile_pool(name="sp", bufs=2) as spp, \
             tc.tile_pool(name="psA", bufs=2, space="PSUM") as psA, \
             tc.tile_pool(name="psB", bufs=2, space="PSUM") as psB, \
             tc.tile_pool(name="psC", bufs=2, space="PSUM") as psC:

            # ---------- constants ----------
            ident = cp.tile([128, 128], F32)
            masks.make_identity(nc, ident[:])
            identb = cp.tile([128, 128], BF16)
            masks.make_identity(nc, identb[:])
            ones_col = cp.tile([128, 1], BF16)
            nc.gpsimd.memset(ones_col[:], 1.0)
            ones1 = cp.tile([1, 128], F32)
            nc.gpsimd.memset(ones1[:], 1.0)

            w1t = cp.tile([128, 64], F32)     # [c-chunk, (avg0 avg1 max0 max1)x16]
            nc.sync.dma_start(w1t[:], w1_d[:])
            w2t = cp.tile([R, C], F32)
            nc.sync.dma_start(w2t[:], w2_d[:])
            b1c = cp.tile([R, 1], F32)
            nc.sync.dma_start(b1c[:], b1_d[:])
            b2r = cp.tile([1, C], F32)
            nc.sync.dma_start(b2r[:], b2_d[:])
            tpt = cp.tile([62, 784], F32)     # 14 Toeplitz mats [62,56]
            nc.sync.dma_start(tpt[:], tp_d[:])
            bcr = cp.tile([1, 1], F32)
            nc.sync.dma_start(bcr[:], bc_d[:])
            bcb = cp.tile([56, 1], F32)
            nc.gpsimd.partition_broadcast(bcb[:], bcr[:], channels=56)

            plane_m = cp.tile([62, 62], F32)
            plane_x = cp.tile([62, 62], F32)
            nc.vector.memset(plane_m[:], 0.0)
            nc.vector.memset(plane_x[:], 0.0)

            # resident x in bf16 (becomes xg, then out, in place)
            xbuf = xp.tile([128, BS * NT * C], BF16)

            for s in range(BS):
                s0 = s * NT * C
                # garbage rows of the 64-row tail tile stay zero throughout
                nc.vector.memset(xbuf[64:128, s0 + 24 * C:s0 + 25 * C], 0.0)

            for s in range(BS):
                s0 = s * NT * C
                vfull = xbuf[:, s0:s0 + NT * C]
                vmain = xbuf[:, s0:s0 + 24 * C]
                vtail = xbuf[0:64, s0 + 24 * C:s0 + 25 * C]

                # ---------- load (cast fp32 -> bf16 during DMA) ----------
                nc.gpsimd.dma_start(
                    vmain, bass.AP(x_d, s * NP * C, [[256, 128], [32768, 24], [1, 256]]))
                nc.gpsimd.dma_start(
                    vtail, bass.AP(x_d, (s * NP + 3072) * C, [[256, 64], [1, 256]]))

                # ---------- phase A: pooling stats ----------
                bankA = psA.tile([128, 512], F32, tag="bankA")
                prow_ps = bankA[0:1, 0:C]
                for t in range(NT):
                    nc.tensor.matmul(
                        prow_ps, ones_col[:],
                        xbuf[:, s0 + t * C:s0 + (t + 1) * C],
                        start=(t == 0), stop=(t == NT - 1),
                        skip_group_check=True)

                macc2 = wp.tile([128, 2 * C], BF16, tag="macc2")
                nc.vector.tensor_tensor(
                    out=macc2[:], in0=xbuf[:, s0:s0 + 2 * C],
                    in1=xbuf[:, s0 + 2 * C:s0 + 4 * C], op=AL.max)
                for q in range(2, 12):
                    nc.vector.tensor_tensor(
                        out=macc2[:], in0=macc2[:],
                        in1=xbuf[:, s0 + 2 * q * C:s0 + 2 * (q + 1) * C],
                        op=AL.max)
                maxacc = wp.tile([128, C], F32, tag="maxacc")
                nc.vector.tensor_tensor(
                    out=maxacc[:], in0=macc2[:, 0:C], in1=macc2[:, C:2 * C],
                    op=AL.max)
                nc.vector.tensor_tensor(
                    out=maxacc[0:64, :], in0=maxacc[0:64, :], in1=vtail, op=AL.max)

                prow = wp.tile([1, C], F32, tag="prow_sb")
                nc.scalar.copy(prow[:], prow_ps)
                pcols_ps = bankA[0:128, 256:260]
                nc.tensor.transpose(pcols_ps[:, 0:1], prow[0:1, 0:128],
                                    ident[0:1, 0:1])
                nc.tensor.transpose(pcols_ps[:, 1:2], prow[0:1, 128:256],
                                    ident[0:1, 0:1])
                bankB = psB.tile([128, 512], F32, tag="bankB")
                mt0_ps = bankB[0:128, 0:128]
                nc.tensor.transpose(mt0_ps, maxacc[:, 0:128], ident[:])
                mt1_ps = bankB[0:128, 128:256]
                nc.tensor.transpose(mt1_ps, maxacc[:, 128:256], ident[:])
                pcols = wp.tile([128, 4], F32, tag="pcols_sb")
                nc.scalar.copy(pcols[:, 0:2], pcols_ps[:, 0:2])
                nc.vector.reduce_max(pcols[:, 2:3], mt0_ps, axis=AX.X)
                nc.vector.reduce_max(pcols[:, 3:4], mt1_ps, axis=AX.X)

                # ---------- phase B: MLP -> per-channel scale ----------
                hT_ps = bankA[0:R, 260:262]
                for p in range(2):          # 0 = avg (w1/NP), 1 = max
                    for k in range(2):      # channel chunk
                        nc.tensor.matmul(
                            hT_ps[:, p:p + 1],
                            w1t[:, (p * 2 + k) * R:(p * 2 + k + 1) * R],
                            pcols[:, 2 * p + k:2 * p + k + 1],
                            start=(k == 0), stop=(k == 1),
                            skip_group_check=True)
                hr = wp.tile([R, 2], F32, tag="hr")
                nc.scalar.activation(hr[:], hT_ps, AF.Relu, bias=b1c[:])
                co_ps = bankA[0:1, 0:2 * C]
                nc.tensor.matmul(co_ps[:, 0:C], hr[:, 0:1], w2t[:],
                                 start=True, stop=True)
                nc.tensor.matmul(co_ps[:, C:2 * C], hr[:, 1:2], w2t[:],
                                 start=True, stop=True)
                sg0 = wp.tile([1, C], F32, tag="sg0")
                nc.vector.tensor_tensor(out=sg0[:], in0=co_ps[0:1, 0:C],
                                        in1=b2r[:], op=AL.add)
                sg1 = wp.tile([1, C], F32, tag="sg1")
                nc.vector.tensor_tensor(out=sg1[:], in0=co_ps[0:1, C:2 * C],
                                        in1=b2r[:], op=AL.add)
                sgs0 = wp.tile([1, C], F32, tag="sgs0")
                nc.scalar.activation(sgs0[:], sg0[:], AF.Sigmoid)
                sgs1 = wp.tile([1, C], F32, tag="sgs1")
                nc.scalar.activation(sgs1[:], sg1[:], AF.Sigmoid)
                bankC = psC.tile([128, 512], F32, tag="bankC")
                cb_ps = bankC[0:128, 0:C]
                nc.tensor.matmul(cb_ps, ones1[:], sgs0[:],
                                 start=True, stop=False, skip_group_check=True)
                nc.tensor.matmul(cb_ps, ones1[:], sgs1[:],
                                 start=False, stop=True, skip_group_check=True)
                cb = wp.tile([128, C], BF16, tag="cb_sb")
                nc.scalar.copy(cb[:], cb_ps)

                # ---------- phase C: xg (in place) + spatial stats ----------
                spx = spp.tile([128, NT], F32, tag="spx")
                spm = spp.tile([128, NT], F32, tag="spm")
                for t in range(NT):
                    vt = xbuf[:, s0 + t * C:s0 + (t + 1) * C]
                    nc.vector.tensor_tensor(out=vt, in0=vt, in1=cb[:],
                                            op=AL.mult)
                    nc.vector.reduce_max(spx[:, t:t + 1], vt, axis=AX.X)
                    nc.scalar.activation(vt, vt, AF.Copy,
                                         accum_out=spm[:, t:t + 1])

                # ---------- phase D: 7x7x2 conv via Toeplitz matmuls ----------
                for c, (sp_t, plane) in enumerate(
                        ((spm, plane_m), (spx, plane_x))):
                    row = s * 2 + c
                    spT_ps = bankB[0:NT, 256 + c * 128:256 + (c + 1) * 128]
                    nc.tensor.transpose(spT_ps, sp_t[:], ident[:])
                    spT = wp.tile([NT, 128], F32, tag="spT_sb")
                    nc.scalar.copy(spT[:], spT_ps)
                    nc.sync.dma_start(
                        bass.AP(flat_dram, row * NP, [[128, 24], [1, 128]]),
                        spT[0:24, :])
                    nc.sync.dma_start(
                        bass.AP(flat_dram, row * NP + 3072, [[1, 64]]),
                        spT[24:25, 0:64])
                    nc.sync.dma_start(
                        plane[3:59, 3:59],
                        bass.AP(flat_dram, row * NP, [[56, 56], [1, 56]]))

                conv_ps = bankC[0:56, 256:312]
                for c, plane in enumerate((plane_m, plane_x)):
                    for dx in range(7):
                        j = (c * 7 + dx) * 56
                        nc.tensor.matmul(
                            conv_ps, tpt[:, j:j + 56], plane[0:62, dx:dx + 56],
                            start=(c == 0 and dx == 0),
                            stop=(c == 1 and dx == 6),
                            skip_group_check=True)
                s_plane = wp.tile([56, 56], F32, tag="splane")
                nc.scalar.activation(s_plane[:], conv_ps, AF.Sigmoid,
                                     bias=bcb[:])
                nc.sync.dma_start(
                    bass.AP(sflat_dram, s * NP, [[1, NP]]), s_plane[:])
                spscT = wp.tile([NT, 128], F32, tag="spscT")
                nc.sync.dma_start(spscT[0:24, :],
                                  bass.AP(sflat_dram, s * NP, [[1, 3072]]))
                nc.sync.dma_start(spscT[24:25, 0:64],
                                  bass.AP(sflat_dram, s * NP + 3072, [[1, 64]]))
                spsc_ps = bankC[0:128, 312:312 + NT]
                nc.tensor.transpose(spsc_ps, spscT[:], ident[0:NT, 0:NT])
                spsc = spp.tile([128, NT], F32, tag="spsc_sb")
                nc.scalar.copy(spsc[:], spsc_ps)

                # ---------- phase E: out = xg * spatial (split engines) ----------
                for t in range(NT):
                    vt = xbuf[:, s0 + t * C:s0 + (t + 1) * C]
                    if t % 2 == 0:
                        nc.vector.tensor_scalar(
                            out=vt, in0=vt, scalar1=spsc[:, t:t + 1],
                            scalar2=None, op0=AL.mult)
                    else:
                        nc.scalar.activation(vt, vt, AF.Copy,
                                             scale=spsc[:, t:t + 1])

                # ---------- store (cast bf16 -> fp32 during DMA) ----------
                nc.gpsimd.dma_start(
                    bass.AP(out_d, s * NP * C, [[256, 128], [32768, 24], [1, 256]]),
                    vmain)
                nc.gpsimd.dma_start(
                    bass.AP(out_d, (s * NP + 3072) * C, [[256, 64], [1, 256]]),
                    vtail)

    nc.compile()
    return nc


def _get_compiled():
    global _COMPILED
    if _COMPILED is None:
        _COMPILED = _build()
    return _COMPILED


def make_in_maps(x, w1, b1, w2, b2, wconv, bconv):
    x = np.ascontiguousarray(np.asarray(x, dtype=np.float32))
    w1 = np.asarray(w1, np.float32)
    w2 = np.asarray(w2, np.float32)

    # w1s: [c-chunk(128), (avg_chunk0, avg_chunk1, max_chunk0, max_chunk1) x 16]
    w1avg = w1 / NP
    w1s = np.concatenate(
        [w1avg[0:128], w1avg[128:256], w1[0:128], w1[128:256]],
        axis=1).astype(np.float32)

    # Toeplitz conv mats: tp[y', (c*7+dx)*56 + y] = wf[y'-y, dx, c]
    wf = np.asarray(wconv, np.float32)[:, :, :, 0]      # [dy, dx, c]
    wf = wf.copy()
    wf[:, :, 0] /= C          # fold channel-mean into mean-plane taps
    tp = np.zeros((2, 7, 62, 56), np.float32)
    idx = np.arange(56)
    for c in range(2):
        for dx in range(7):
            for dy in range(7):
                tp[c, dx, idx + dy, idx] = wf[dy, dx, c]
    tp = tp.transpose(2, 0, 1, 3).reshape(62, 784).copy()

    xs = x.reshape(N_CORES, ROWS, C)
    return [{
        "x": xs[i],
        "w1s": w1s,
        "w2": w2,
        "b1c": np.asarray(b1, np.float32).reshape(R, 1),
        "b2r": np.asarray(b2, np.float32).reshape(1, C),
        "tp": tp,
        "bconv": np.asarray(bconv, np.float32).reshape(1, 1),
    } for i in range(N_CORES)]


def kernel(x, w1, b1, w2, b2, wconv, bconv):
    nc = _get_compiled()
    in_maps = make_in_maps(x, w1, b1, w2, b2, wconv, bconv)
    res = run_bass_kernel_spmd(nc, in_maps, list(range(N_CORES)))
    out = np.stack([res.results[i]["out"] for i in range(N_CORES)], axis=0)
    return out.reshape(B, H, W, C)
